# revision 9
# baseline (speedup 1.0000x reference)
"""Trainium2 Bass kernel for one pre-LN transformer block (B=8, T=1024, C=256,
H=16 heads of size 16, FFN 256->1024->256), data-parallel over batch across 8
NeuronCores (one batch element per core).

Per-core dataflow (all matmul operands bf16, accumulation fp32):
  LN1 (straight [T,C]) -> PE-transpose -> h1^T [C,T]
  Q^T/K^T in padded head layout [h*32+d, T] (pad rows zero, from padded weights)
  V straight [T, h*32+{d,16=ones-col,zeros}] - the ones column makes the PV
    matmul also produce the softmax denominator (scores are small: no max pass)
  S^T[tk,tq] = k^T.T @ q^T per head via 32-row-strip matmuls (4 heads share the
    128-row PE array), exp fused into the PSUM->SBUF copy on ScalarE,
    causal diag blocks masked by a triangular multiply
  PV: out^T[d,tq] accumulated over tk tiles with 32-col-strip matmuls
  normalize via per-head Z row broadcast (stream_shuffle) + reciprocal + mult
  proj: x1 = x(+bp) + out^T.T @ Wp   (out^T tiles are the stationary operand)
  LN2 -> h2^T -> FFN1 (relu+bias fused in ScalarE evac) -> FFN2 -> + x1
"""

import os
import sys

for _p in ("/opt/trn_rl_repo", "/root/.axon_site/_ro/trn_rl_repo"):
    if os.path.isdir(_p) and _p not in sys.path:
        sys.path.append(_p)

import numpy as np
import ml_dtypes

# problem shapes (hardcoded per contest rules)
B, T, C, H, D, F = 8, 1024, 256, 16, 16, 1024
P = 128          # partitions
NT = T // P      # 8 T-tiles
HP = 32          # padded per-head stride (Q/K/V/out layouts)
CP = H * HP      # 512 padded channel dim
NPACK = 4        # head packs (4 heads per 128-partition tile)
NKC = C // P     # 2 k-tiles over C
EPS = 1e-5
SCALE = D ** -0.5

_BF16 = ml_dtypes.bfloat16

_cache = {}


def _build_program():
    import concourse.bass as bass
    import concourse.bacc as bacc
    import concourse.tile as tile
    import concourse.mybir as mybir

    dt = mybir.dt
    f32, bf16 = dt.float32, dt.bfloat16
    AF = mybir.ActivationFunctionType
    ALU = mybir.AluOpType

    nc = bacc.Bacc("TRN2", target_bir_lowering=False, debug=False)

    # ---- DRAM I/O ----
    x_d = nc.dram_tensor("x", [T, C], f32, kind="ExternalInput")
    wq_d = nc.dram_tensor("wq", [C, CP], bf16, kind="ExternalInput")
    wk_d = nc.dram_tensor("wk", [C, CP], bf16, kind="ExternalInput")
    wv_d = nc.dram_tensor("wv", [C, CP], bf16, kind="ExternalInput")
    wp_d = nc.dram_tensor("wp", [CP, C], bf16, kind="ExternalInput")
    w1_d = nc.dram_tensor("w1", [C, F], bf16, kind="ExternalInput")
    w2_d = nc.dram_tensor("w2", [F, C], bf16, kind="ExternalInput")
    bq_d = nc.dram_tensor("bq", [CP], f32, kind="ExternalInput")
    bk_d = nc.dram_tensor("bk", [CP], f32, kind="ExternalInput")
    bv_d = nc.dram_tensor("bv", [CP], f32, kind="ExternalInput")
    bp_d = nc.dram_tensor("bprow", [C], f32, kind="ExternalInput")
    b1_d = nc.dram_tensor("b1p", [F], f32, kind="ExternalInput")
    b2_d = nc.dram_tensor("b2row", [C], f32, kind="ExternalInput")
    out_d = nc.dram_tensor("out", [T, C], f32, kind="ExternalOutput")

    ident_np = np.eye(P, dtype=_BF16)
    # S^T diag tile mask: partition = tk local, free = tq local; keep tq >= tk
    tri_np = np.triu(np.ones((P, P), dtype=np.float32)).astype(_BF16)
    ident_d = nc.inline_tensor(ident_np, name="ident")
    tri_d = nc.inline_tensor(tri_np, name="trimask")

    with tile.TileContext(nc) as tc:
        consts = tc.alloc_tile_pool(name="consts", bufs=1)
        data = tc.alloc_tile_pool(name="data", bufs=1)
        attn = tc.alloc_tile_pool(name="attn", bufs=1)
        work = tc.alloc_tile_pool(name="work", bufs=3)
        psum = tc.alloc_tile_pool(name="psum", bufs=1, space="PSUM")

        # ---- persistent SBUF tensors ----
        ident_s = consts.tile([P, P], bf16)
        tri_s = consts.tile([P, P], bf16)
        eps_s = consts.tile([P, 1], f32)
        wq_s = consts.tile([P, NKC, CP], bf16)
        wk_s = consts.tile([P, NKC, CP], bf16)
        wv_s = consts.tile([P, NKC, CP], bf16)
        wp_s = consts.tile([P, NPACK, C], bf16)
        w1_s = consts.tile([P, NKC, F], bf16)
        w2_s = consts.tile([P, NT, C], bf16)
        bq_s = consts.tile([P, NPACK], f32)
        bk_s = consts.tile([P, NPACK], f32)
        b1_s = consts.tile([P, NT], f32)

        xs = data.tile([P, NT, C], f32)
        xbp = data.tile([P, NT, C], f32)
        h1T = data.tile([P, NKC, T], bf16)
        QT = data.tile([P, NPACK, T], bf16)
        KT = data.tile([P, NPACK, T], bf16)
        Vv = data.tile([P, NT, CP], bf16)
        OUTT = data.tile([P, NPACK, T], bf16)
        x1 = data.tile([P, NT, C], f32)
        h2T = data.tile([P, NKC, T], bf16)
        HT = data.tile([P, NT, F], bf16)

        # ---- input DMAs ----
        nc.sync.dma_start(out=ident_s, in_=ident_d[:, :])
        nc.sync.dma_start(out=tri_s, in_=tri_d[:, :])
        nc.vector.memset(eps_s, EPS)
        nc.sync.dma_start(out=xs, in_=x_d[:, :].rearrange("(j p) c -> p j c", p=P))
        nc.sync.dma_start(out=xbp, in_=x_d[:, :].rearrange("(j p) c -> p j c", p=P))
        # add bp (broadcast along partitions) into the residual copy
        bp_b = bass.AP(tensor=bp_d, offset=0, ap=[[0, P], [1, C]])
        bpt = consts.tile([P, C], f32)
        nc.sync.dma_start(out=bpt, in_=bp_b)
        for j in range(NT):
            nc.vector.tensor_add(out=xbp[:, j], in0=xbp[:, j], in1=bpt)
        nc.sync.dma_start(out=wq_s, in_=wq_d[:, :].rearrange("(k p) c -> p k c", p=P))
        nc.sync.dma_start(out=wk_s, in_=wk_d[:, :].rearrange("(k p) c -> p k c", p=P))
        nc.sync.dma_start(out=wv_s, in_=wv_d[:, :].rearrange("(k p) c -> p k c", p=P))
        nc.sync.dma_start(out=wp_s, in_=wp_d[:, :].rearrange("(k p) c -> p k c", p=P))
        nc.sync.dma_start(out=w1_s, in_=w1_d[:, :].rearrange("(k p) c -> p k c", p=P))
        nc.sync.dma_start(out=w2_s, in_=w2_d[:, :].rearrange("(k p) c -> p k c", p=P))
        nc.sync.dma_start(out=bq_s, in_=bq_d[:].rearrange("(m p) -> p m", p=P))
        nc.sync.dma_start(out=bk_s, in_=bk_d[:].rearrange("(m p) -> p m", p=P))
        nc.sync.dma_start(out=b1_s, in_=b1_d[:].rearrange("(m p) -> p m", p=P))
        bv_s = consts.tile([P, NPACK], f32)
        nc.sync.dma_start(out=bv_s, in_=bv_d[:].rearrange("(m p) -> p m", p=P))
        b2t = consts.tile([P, C], f32)
        b2_b = bass.AP(tensor=b2_d, offset=0, ap=[[0, P], [1, C]])
        nc.sync.dma_start(out=b2t, in_=b2_b)

        def layernorm(src, j, dst_h, dst_hT):
            """src[:, j] [128, 256] f32 -> dst_h [128,256] bf16 and its
            transpose into dst_hT[:, :, j*128:(j+1)*128]."""
            stats = work.tile([P, 6], f32, tag="stats")
            mv = work.tile([P, 2], f32, tag="mv")
            nc.vector.bn_stats(out=stats, in_=src[:, j])
            nc.vector.bn_aggr(out=mv, in_=stats)
            lnv = work.tile([P, 1], f32, tag="lnv")
            rstd = work.tile([P, 1], f32, tag="rstd")
            # rstd = exp(-0.5*ln(var+eps)) : keeps ACT on one table set (ln/exp)
            nc.scalar.activation(out=lnv, in_=mv[:, 1:2], func=AF.Ln, bias=eps_s)
            nc.scalar.activation(out=rstd, in_=lnv, func=AF.Exp, scale=-0.5)
            nc.vector.tensor_scalar(
                out=dst_h, in0=src[:, j],
                scalar1=mv[:, 0:1], scalar2=rstd,
                op0=ALU.subtract, op1=ALU.mult,
            )
            tp = psum.tile([P, 2, P], bf16, tag="mm256", bufs=2)
            nc.tensor.transpose(tp[:, 0], dst_h[:, 0:P], ident_s)
            nc.tensor.transpose(tp[:, 1], dst_h[:, P : 2 * P], ident_s)
            nc.vector.tensor_copy(dst_hT[:, :, j * P : (j + 1) * P], tp)

        # ---- LN1 + h1^T ----
        for j in range(NT):
            h1s = work.tile([P, C], bf16, tag="hstraight")
            layernorm(xs, j, h1s, h1T)

        # ---- Q^T / K^T (padded layout, bias folded in evac) ----
        for (name, w_s, b_s, dstT) in (("q", wq_s, bq_s, QT), ("k", wk_s, bk_s, KT)):
            for m in range(NPACK):
                for c in range(2):
                    ps = psum.tile([P, 512], f32, tag="mm512", bufs=6)
                    for k in range(NKC):
                        nc.tensor.matmul(
                            ps,
                            lhsT=w_s[:, k, m * P : (m + 1) * P],
                            rhs=h1T[:, k, c * 512 : (c + 1) * 512],
                            start=(k == 0), stop=(k == NKC - 1),
                        )
                    nc.vector.tensor_scalar_add(
                        out=dstT[:, m, c * 512 : (c + 1) * 512], in0=ps,
                        scalar1=b_s[:, m : m + 1],
                    )

        # ---- V (straight, padded 32-wide blocks; col 16 of each = ones) ----
        for j in range(NT):
            ps = psum.tile([P, 512], f32, tag="mm512", bufs=6)
            for k in range(NKC):
                nc.tensor.matmul(
                    ps,
                    lhsT=h1T[:, k, j * P : (j + 1) * P],
                    rhs=wv_s[:, k, :],
                    start=(k == 0), stop=(k == NKC - 1),
                )
            nc.scalar.copy(Vv[:, j, :], ps)
        ones_cols = Vv.rearrange("p j (h e) -> p j h e", e=HP)[:, :, :, 16:17]
        nc.vector.memset(ones_cols, 1.0)

        # ---- attention, one pack (4 heads) at a time ----
        for p in range(NPACK):
            expb = attn.tile([P, NPACK, NT, T], bf16, tag="expb", bufs=1)
            # S^T + exp per tk-tile
            for i in range(NT):
                lo = P * i
                segs = []
                if lo < 512:
                    segs.append((lo, 512 - lo))
                    segs.append((512, 512))
                else:
                    segs.append((lo, T - lo))
                for (o, n) in segs:
                    sps = []
                    for hh in range(NPACK):
                        sp = psum.tile([P, 512], f32, tag="mm512", bufs=6,
                                       name=f"sp{p}_{i}_{o}_{hh}")
                        nc.tensor.matmul(
                            sp[:, 0:n],
                            lhsT=KT[HP * hh : HP * (hh + 1), p, i * P : (i + 1) * P],
                            rhs=QT[HP * hh : HP * (hh + 1), p, o : o + n],
                            start=True, stop=True,
                            tile_position=(HP * hh, 0),
                        )
                        sps.append(sp)
                    for hh in range(NPACK):
                        nc.scalar.activation(
                            out=expb[:, hh, i, o : o + n], in_=sps[hh][:, 0:n],
                            func=AF.Exp, scale=SCALE,
                        )
                # causal mask on the diagonal block
                for hh in range(NPACK):
                    nc.vector.tensor_mul(
                        out=expb[:, hh, i, lo : lo + P],
                        in0=expb[:, hh, i, lo : lo + P],
                        in1=tri_s,
                    )
            # PV per tq chunk
            for cj in range(2):
                pv = psum.tile([P, 512], f32, tag="mm512", bufs=6,
                               name=f"pv{p}_{cj}")
                tiles = range(0, min(NT, 4 * cj + 4))
                last = max(tiles)
                for i in tiles:
                    off = max(0, P * i - 512 * cj)
                    n = 512 - off
                    for hh in range(NPACK):
                        h = 4 * p + hh
                        nc.tensor.matmul(
                            pv[HP * hh : HP * (hh + 1), off : off + n],
                            lhsT=Vv[:, i, HP * h : HP * (h + 1)],
                            rhs=expb[:, hh, i, 512 * cj + off : 512 * cj + off + n],
                            start=(i == 0), stop=(i == last),
                            tile_position=(0, HP * hh),
                            skip_group_check=True,
                        )
                # normalize: out^T = pv / Z  (Z in partition 16 of each 32-block)
                pvraw = work.tile([P, 512], bf16, tag="pvraw")
                zbc = work.tile([P, 512], bf16, tag="zbc")
                rz = work.tile([P, 512], f32, tag="rz")
                nc.scalar.copy(pvraw, pv)
                nc.vector.stream_shuffle(zbc, pvraw, mask=[16] * 32)
                nc.vector.reciprocal(rz, zbc)
                nc.vector.tensor_mul(
                    out=OUTT[:, p, 512 * cj : 512 * (cj + 1)], in0=pvraw, in1=rz
                )

        # ---- attention out-projection + residual ----
        for j in range(NT):
            ps = psum.tile([P, C], f32, tag="mm256", bufs=2)
            for k in range(NPACK):
                nc.tensor.matmul(
                    ps,
                    lhsT=OUTT[:, k, j * P : (j + 1) * P],
                    rhs=wp_s[:, k, :],
                    start=(k == 0), stop=(k == NPACK - 1),
                )
            nc.vector.tensor_add(out=x1[:, j], in0=ps, in1=xbp[:, j])

        # ---- LN2 + h2^T ----
        for j in range(NT):
            h2s = work.tile([P, C], bf16, tag="hstraight")
            layernorm(x1, j, h2s, h2T)

        # ---- FFN1: H^T = relu(W1^T h2^T + b1) ----
        for f in range(NT):
            for c in range(2):
                ps = psum.tile([P, 512], f32, tag="mm512", bufs=6)
                for k in range(NKC):
                    nc.tensor.matmul(
                        ps,
                        lhsT=w1_s[:, k, f * P : (f + 1) * P],
                        rhs=h2T[:, k, c * 512 : (c + 1) * 512],
                        start=(k == 0), stop=(k == NKC - 1),
                    )
                nc.scalar.activation(
                    out=HT[:, f, c * 512 : (c + 1) * 512], in_=ps,
                    func=AF.Relu, bias=b1_s[:, f : f + 1],
                )

        # ---- FFN2 + residual + b2, write out ----
        for j in range(NT):
            ps = psum.tile([P, C], f32, tag="mm256", bufs=2)
            for f in range(NT):
                nc.tensor.matmul(
                    ps,
                    lhsT=HT[:, f, j * P : (j + 1) * P],
                    rhs=w2_s[:, f, :],
                    start=(f == 0), stop=(f == NT - 1),
                )
            outs = work.tile([P, C], f32, tag="outs")
            nc.vector.tensor_add(out=outs, in0=ps, in1=x1[:, j])
            nc.vector.tensor_add(out=outs, in0=outs, in1=b2t)
            nc.sync.dma_start(
                out=out_d[:, :].rearrange("(t p) c -> p t c", p=P)[:, j], in_=outs
            )

        for pool in (psum, work, attn, data, consts):
            pool.release()

    nc.compile()
    return nc


def _prep_inputs(x, Wq, Wk, Wv, Wp, bp, W1, b1, W2, b2, g1, be1, g2, be2):
    """Host-side preprocessing: fold LN affines into the following matmuls,
    pad per-head weights to 32-wide blocks, cast to bf16."""
    f32 = np.float32
    x = np.asarray(x, f32)
    Wqf = np.asarray(Wq, f32).reshape(C, C) * np.asarray(g1, f32)[:, None]
    Wkf = np.asarray(Wk, f32).reshape(C, C) * np.asarray(g1, f32)[:, None]
    Wvf = np.asarray(Wv, f32).reshape(C, C) * np.asarray(g1, f32)[:, None]
    bqf = np.asarray(be1, f32) @ np.asarray(Wq, f32).reshape(C, C)
    bkf = np.asarray(be1, f32) @ np.asarray(Wk, f32).reshape(C, C)
    bvf = np.asarray(be1, f32) @ np.asarray(Wv, f32).reshape(C, C)

    def pad_cols(w):
        wp = np.zeros((C, CP), f32)
        for h in range(H):
            wp[:, HP * h : HP * h + D] = w[:, D * h : D * (h + 1)]
        return wp

    def pad_vec(v):
        vp = np.zeros((CP,), f32)
        for h in range(H):
            vp[HP * h : HP * h + D] = v[D * h : D * (h + 1)]
        return vp

    wq_p = pad_cols(Wqf)
    wk_p = pad_cols(Wkf)
    wv_p = pad_cols(Wvf)
    bq_p = pad_vec(bqf)
    bk_p = pad_vec(bkf)
    bv_p = pad_vec(bvf)

    wp_p = np.zeros((CP, C), f32)
    for h in range(H):
        wp_p[HP * h : HP * h + D, :] = np.asarray(Wp, f32)[D * h : D * (h + 1), :]

    W1f = np.asarray(W1, f32) * np.asarray(g2, f32)[:, None]
    b1f = np.asarray(b1, f32) + np.asarray(be2, f32) @ np.asarray(W1, f32)

    shared = {
        "wq": wq_p.astype(_BF16), "wk": wk_p.astype(_BF16),
        "wv": wv_p.astype(_BF16), "wp": wp_p.astype(_BF16),
        "w1": W1f.astype(_BF16), "w2": np.asarray(W2, f32).astype(_BF16),
        "bq": bq_p, "bk": bk_p, "bv": bv_p,
        "bprow": np.asarray(bp, f32), "b1p": b1f,
        "b2row": np.asarray(b2, f32),
    }
    assert not np.any(bv_p), "nonzero V bias not folded on-device (be1 != 0)"
    return x, shared


def kernel(**inputs) -> np.ndarray:
    from concourse import bass_utils

    x, shared = _prep_inputs(**inputs)
    if "nc" not in _cache:
        _cache["nc"] = _build_program()
    nc = _cache["nc"]

    in_maps = [dict(shared, x=np.ascontiguousarray(x[i])) for i in range(B)]
    res = bass_utils.run_bass_kernel_spmd(nc, in_maps, core_ids=list(range(B)))
    _cache["last_result"] = res
    out = np.stack([r["out"] for r in res.results], axis=0)
    return out.astype(np.float32)


# revision 11
# speedup vs baseline: 1.1543x; 1.1543x over previous
"""Trainium2 Bass kernel for one pre-LN transformer block (B=8, T=1024, C=256,
H=16 heads of size 16, FFN 256->1024->256), data-parallel over batch across 8
NeuronCores (one batch element per core).

Per-core dataflow (all matmul operands bf16, accumulation fp32):
  LN1 (straight [T,C], rstd via batched Quake-rsqrt on DVE) -> PE-transpose ->
    h1^T [C,T]
  Q^T/K^T in padded head layout [h*32+d, T] (pad rows zero, padded weights)
  V straight [T, h*32+{d,16=ones-col,zeros}] - the ones column makes the PV
    matmul also produce the softmax denominator (scores are tiny: no max pass)
  S^T[tk,tq] = k^T.T @ q^T per head via 32-row-strip matmuls (4 heads share
    the 128-row PE array), exp fused into the PSUM->SBUF copy on ScalarE
    (2 heads per activation op), causal diag blocks masked by one batched
    diagonal-strided triangular multiply per (pack, head)
  PV: out^T[d,tq] accumulated over tk tiles with 32-col-strip matmuls
  normalize via per-head Z row broadcast (stream_shuffle from PSUM) +
    reciprocal_approx_fast + multiply
  proj: x1 = x(+bp) + out^T.T @ Wp   (out^T tiles are the stationary operand)
  LN2 -> h2^T -> FFN1 (relu+bias on DVE evac) -> FFN2 -> + x1
"""

import os
import sys

for _p in ("/opt/trn_rl_repo", "/root/.axon_site/_ro/trn_rl_repo"):
    if os.path.isdir(_p) and _p not in sys.path:
        sys.path.append(_p)

import numpy as np
import ml_dtypes

# problem shapes (hardcoded per contest rules)
B, T, C, H, D, F = 8, 1024, 256, 16, 16, 1024
P = 128          # partitions
NT = T // P      # 8 T-tiles
HP = 32          # padded per-head stride (Q/K/V/out layouts)
CP = H * HP      # 512 padded channel dim
NPACK = 4        # head packs (4 heads per 128-partition tile)
NKC = C // P     # 2 k-tiles over C
EPS = 1e-5
SCALE = D ** -0.5
MAGIC = 0x5F3759DF

_BF16 = ml_dtypes.bfloat16

_cache = {}


def _build_program():
    import concourse.bass as bass
    import concourse.bacc as bacc
    import concourse.tile as tile
    import concourse.mybir as mybir

    dt = mybir.dt
    f32, bf16, i32 = dt.float32, dt.bfloat16, dt.int32
    AF = mybir.ActivationFunctionType
    ALU = mybir.AluOpType

    nc = bacc.Bacc("TRN2", target_bir_lowering=False, debug=False)

    # ---- DRAM I/O ----
    x_d = nc.dram_tensor("x", [T, C], f32, kind="ExternalInput")
    wq_d = nc.dram_tensor("wq", [C, CP], bf16, kind="ExternalInput")
    wk_d = nc.dram_tensor("wk", [C, CP], bf16, kind="ExternalInput")
    wv_d = nc.dram_tensor("wv", [C, CP], bf16, kind="ExternalInput")
    wp_d = nc.dram_tensor("wp", [CP, C], bf16, kind="ExternalInput")
    w1_d = nc.dram_tensor("w1", [C, F], bf16, kind="ExternalInput")
    w2_d = nc.dram_tensor("w2", [F, C], bf16, kind="ExternalInput")
    bq_d = nc.dram_tensor("bq", [CP], f32, kind="ExternalInput")
    bk_d = nc.dram_tensor("bk", [CP], f32, kind="ExternalInput")
    bp_d = nc.dram_tensor("bprow", [C], f32, kind="ExternalInput")
    b1_d = nc.dram_tensor("b1p", [F], f32, kind="ExternalInput")
    b2_d = nc.dram_tensor("b2row", [C], f32, kind="ExternalInput")
    out_d = nc.dram_tensor("out", [T, C], f32, kind="ExternalOutput")

    ident_np = np.eye(P, dtype=_BF16)
    # S^T diag tile mask: partition = tk local, free = tq local; keep tq >= tk
    tri_np = np.triu(np.ones((P, P), dtype=np.float32))
    tri8_np = np.tile(tri_np, (1, NT)).astype(_BF16)  # [128, 8*128]
    ident_d = nc.inline_tensor(ident_np, name="ident")
    tri8_d = nc.inline_tensor(tri8_np, name="trimask8")

    with tile.TileContext(nc) as tc:
        consts = tc.alloc_tile_pool(name="consts", bufs=1)
        data = tc.alloc_tile_pool(name="data", bufs=1)
        attn = tc.alloc_tile_pool(name="attn", bufs=1)
        work = tc.alloc_tile_pool(name="work", bufs=3)
        psum = tc.alloc_tile_pool(name="psum", bufs=1, space="PSUM")

        # ---- persistent SBUF tensors ----
        ident_s = consts.tile([P, P], bf16)
        tri8_s = consts.tile([P, NT * P], bf16)
        wq_s = consts.tile([P, NKC, CP], bf16)
        wk_s = consts.tile([P, NKC, CP], bf16)
        wv_s = consts.tile([P, NKC, CP], bf16)
        wp_s = consts.tile([P, NPACK, C], bf16)
        w1_s = consts.tile([P, NKC, F], bf16)
        w2_s = consts.tile([P, NT, C], bf16)
        bq_s = consts.tile([P, NPACK], f32)
        bk_s = consts.tile([P, NPACK], f32)
        b1_s = consts.tile([P, NT], f32)

        xs = data.tile([P, NT, C], f32)
        xbp = data.tile([P, NT, C], f32)
        h1T = data.tile([P, NKC, T], bf16)
        QT = data.tile([P, NPACK, T], bf16)
        KT = data.tile([P, NPACK, T], bf16)
        Vv = data.tile([P, NT, CP], bf16)
        OUTT = data.tile([P, NPACK, T], bf16)
        x1 = data.tile([P, NT, C], f32)
        h2T = data.tile([P, NKC, T], bf16)
        HT = data.tile([P, NT, F], bf16)

        # ---- input DMAs ----
        nc.sync.dma_start(out=ident_s, in_=ident_d[:, :])
        nc.sync.dma_start(out=tri8_s, in_=tri8_d[:, :])
        nc.sync.dma_start(out=xs, in_=x_d[:, :].rearrange("(j p) c -> p j c", p=P))
        nc.sync.dma_start(out=xbp, in_=x_d[:, :].rearrange("(j p) c -> p j c", p=P))
        # add bp (broadcast along partitions) into the residual copy
        bp_b = bass.AP(tensor=bp_d, offset=0, ap=[[0, P], [1, C]])
        bpt = consts.tile([P, C], f32)
        nc.sync.dma_start(out=bpt, in_=bp_b)
        for j in range(NT):
            nc.vector.tensor_add(out=xbp[:, j], in0=xbp[:, j], in1=bpt)
        nc.sync.dma_start(out=wq_s, in_=wq_d[:, :].rearrange("(k p) c -> p k c", p=P))
        nc.sync.dma_start(out=wk_s, in_=wk_d[:, :].rearrange("(k p) c -> p k c", p=P))
        nc.sync.dma_start(out=wv_s, in_=wv_d[:, :].rearrange("(k p) c -> p k c", p=P))
        nc.sync.dma_start(out=wp_s, in_=wp_d[:, :].rearrange("(k p) c -> p k c", p=P))
        nc.sync.dma_start(out=w1_s, in_=w1_d[:, :].rearrange("(k p) c -> p k c", p=P))
        nc.sync.dma_start(out=w2_s, in_=w2_d[:, :].rearrange("(k p) c -> p k c", p=P))
        nc.sync.dma_start(out=bq_s, in_=bq_d[:].rearrange("(m p) -> p m", p=P))
        nc.sync.dma_start(out=bk_s, in_=bk_d[:].rearrange("(m p) -> p m", p=P))
        nc.sync.dma_start(out=b1_s, in_=b1_d[:].rearrange("(m p) -> p m", p=P))
        b2t = consts.tile([P, C], f32)
        b2_b = bass.AP(tensor=b2_d, offset=0, ap=[[0, P], [1, C]])
        nc.sync.dma_start(out=b2t, in_=b2_b)

        def ln_phase(src, dst_hT, tag):
            """LayerNorm all 8 tiles of src [128, 8, 256] f32 and write the
            transposed bf16 result into dst_hT [128, 2, 1024]."""
            mvall = work.tile([P, NT, 2], f32, tag="mvall", name=f"mv_{tag}")
            for j in range(NT):
                stats = work.tile([P, 6], f32, tag="stats")
                nc.vector.bn_stats(out=stats, in_=src[:, j])
                nc.vector.bn_aggr(out=mvall[:, j], in_=stats)
            # rstd for all tiles: Quake rsqrt + 2 Newton steps (pure DVE)
            vpe = work.tile([P, NT], f32, tag="vpe", name=f"vpe_{tag}")
            nc.vector.tensor_scalar_add(out=vpe, in0=mvall[:, :, 1], scalar1=EPS)
            sh = work.tile([P, NT], i32, tag="rsq_sh")
            nc.vector.tensor_scalar(
                out=sh, in0=vpe.bitcast(i32), scalar1=1, scalar2=None,
                op0=ALU.logical_shift_right,
            )
            y0 = work.tile([P, NT], i32, tag="rsq_y0")
            nc.vector.tensor_scalar(
                out=y0, in0=sh, scalar1=-1, scalar2=MAGIC,
                op0=ALU.mult, op1=ALU.add,
            )
            y = y0.bitcast(f32)
            rsq = work.tile([P, NT], f32, tag="rsq", name=f"rsq_{tag}")
            tmp = work.tile([P, NT], f32, tag="rsq_tmp")
            for it in range(2):
                nc.vector.tensor_tensor(out=tmp, in0=y, in1=y, op=ALU.mult)
                nc.vector.tensor_tensor(out=tmp, in0=tmp, in1=vpe, op=ALU.mult)
                nc.vector.tensor_scalar(
                    out=tmp, in0=tmp, scalar1=-0.5, scalar2=1.5,
                    op0=ALU.mult, op1=ALU.add,
                )
                nc.vector.tensor_tensor(out=rsq, in0=tmp, in1=y, op=ALU.mult)
                y = rsq
            for j in range(NT):
                hs = work.tile([P, C], bf16, tag="hstraight")
                nc.vector.tensor_scalar(
                    out=hs, in0=src[:, j],
                    scalar1=mvall[:, j, 0:1], scalar2=rsq[:, j : j + 1],
                    op0=ALU.subtract, op1=ALU.mult,
                )
                tp = psum.tile([P, 2, P], bf16, tag="mm256", bufs=2)
                nc.tensor.transpose(tp[:, 0], hs[:, 0:P], ident_s)
                nc.tensor.transpose(tp[:, 1], hs[:, P : 2 * P], ident_s)
                nc.vector.tensor_copy(dst_hT[:, :, j * P : (j + 1) * P], tp)

        # ---- LN1 + h1^T ----
        ln_phase(xs, h1T, "ln1")

        # ---- Q^T / K^T (padded layout, bias folded in evac) ----
        for (name, w_s, b_s, dstT) in (("q", wq_s, bq_s, QT), ("k", wk_s, bk_s, KT)):
            for m in range(NPACK):
                for c in range(2):
                    ps = psum.tile([P, 512], f32, tag="pv", bufs=2)
                    for k in range(NKC):
                        nc.tensor.matmul(
                            ps,
                            lhsT=w_s[:, k, m * P : (m + 1) * P],
                            rhs=h1T[:, k, c * 512 : (c + 1) * 512],
                            start=(k == 0), stop=(k == NKC - 1),
                        )
                    nc.vector.tensor_scalar_add(
                        out=dstT[:, m, c * 512 : (c + 1) * 512], in0=ps,
                        scalar1=b_s[:, m : m + 1],
                    )

        # ---- V (straight, padded 32-wide blocks; col 16 of each = ones) ----
        for j in range(NT):
            ps = psum.tile([P, 512], f32, tag="pv", bufs=2)
            for k in range(NKC):
                nc.tensor.matmul(
                    ps,
                    lhsT=h1T[:, k, j * P : (j + 1) * P],
                    rhs=wv_s[:, k, :],
                    start=(k == 0), stop=(k == NKC - 1),
                )
            nc.scalar.copy(Vv[:, j, :], ps)
        ones_cols = Vv.rearrange("p j (h e) -> p j h e", e=HP)[:, :, :, 16:17]
        nc.vector.memset(ones_cols, 1.0)

        # ---- attention, one pack (4 heads) at a time ----
        for p in range(NPACK):
            expb = attn.tile([P, NPACK, NT, T], bf16, tag="expb", bufs=1)
            # S^T + exp per tk-tile; 2 heads share one 2-bank psum tile
            for i in range(NT):
                lo = P * i
                segs = []
                if lo < 512:
                    segs.append((lo, 512 - lo))
                    segs.append((512, 512))
                else:
                    segs.append((lo, T - lo))
                for (o, n) in segs:
                    for q in range(2):  # head pair
                        sp = psum.tile([P, 2, 512], f32, tag="sps", bufs=2,
                                       name=f"sp{p}_{i}_{o}_{q}")
                        for e in range(2):
                            hh = 2 * q + e
                            nc.tensor.matmul(
                                sp[:, e, 0:n],
                                lhsT=KT[HP * hh : HP * (hh + 1), p,
                                        i * P : (i + 1) * P],
                                rhs=QT[HP * hh : HP * (hh + 1), p, o : o + n],
                                start=True, stop=True,
                                tile_position=(HP * hh, 0),
                            )
                        nc.scalar.activation(
                            out=expb[:, 2 * q : 2 * q + 2, i, o : o + n],
                            in_=sp[:, :, 0:n],
                            func=AF.Exp, scale=SCALE,
                        )
            # causal mask on all 8 diagonal blocks per head: one strided op
            for hh in range(NPACK):
                base = expb[:, hh]
                dview = bass.AP(
                    tensor=base.tensor,
                    offset=base.offset,
                    ap=[list(base.ap[0]), [T + P, NT], [1, P]],
                )
                nc.vector.tensor_tensor(
                    out=dview, in0=dview,
                    in1=tri8_s[:, :].rearrange("p (j q) -> p j q", q=P),
                    op=ALU.mult,
                )
            # PV per tq chunk
            for cj in range(2):
                pv = psum.tile([P, 512], f32, tag="pv", bufs=2,
                               name=f"pv{p}_{cj}")
                tiles = range(0, min(NT, 4 * cj + 4))
                last = max(tiles)
                for i in tiles:
                    off = max(0, P * i - 512 * cj)
                    n = 512 - off
                    for hh in range(NPACK):
                        h = 4 * p + hh
                        nc.tensor.matmul(
                            pv[HP * hh : HP * (hh + 1), off : off + n],
                            lhsT=Vv[:, i, HP * h : HP * (h + 1)],
                            rhs=expb[:, hh, i, 512 * cj + off : 512 * cj + off + n],
                            start=(i == 0), stop=(i == last),
                            tile_position=(0, HP * hh),
                            skip_group_check=True,
                        )
                # normalize: out^T = pv / Z  (Z in partition 16 of each 32-block)
                zbc = work.tile([P, 512], f32, tag="zbc")
                rz = work.tile([P, 512], f32, tag="rz")
                nc.vector.stream_shuffle(zbc, pv, mask=[16] * 32)
                nc.vector.reciprocal_approx_fast(out=rz, in_=zbc)
                nc.vector.tensor_tensor(
                    out=OUTT[:, p, 512 * cj : 512 * (cj + 1)], in0=pv, in1=rz,
                    op=ALU.mult,
                )

        # ---- attention out-projection + residual ----
        for j in range(NT):
            ps = psum.tile([P, C], f32, tag="mm256", bufs=2)
            for k in range(NPACK):
                nc.tensor.matmul(
                    ps,
                    lhsT=OUTT[:, k, j * P : (j + 1) * P],
                    rhs=wp_s[:, k, :],
                    start=(k == 0), stop=(k == NPACK - 1),
                )
            nc.vector.tensor_add(out=x1[:, j], in0=ps, in1=xbp[:, j])

        # ---- LN2 + h2^T ----
        ln_phase(x1, h2T, "ln2")

        # ---- FFN1: H^T = relu(W1^T h2^T + b1) on DVE evac ----
        for f in range(NT):
            for c in range(2):
                ps = psum.tile([P, 512], f32, tag="pv", bufs=2)
                for k in range(NKC):
                    nc.tensor.matmul(
                        ps,
                        lhsT=w1_s[:, k, f * P : (f + 1) * P],
                        rhs=h2T[:, k, c * 512 : (c + 1) * 512],
                        start=(k == 0), stop=(k == NKC - 1),
                    )
                nc.vector.tensor_scalar(
                    out=HT[:, f, c * 512 : (c + 1) * 512], in0=ps,
                    scalar1=b1_s[:, f : f + 1], scalar2=0.0,
                    op0=ALU.add, op1=ALU.max,
                )

        # ---- FFN2 + residual + b2, write out ----
        for j in range(NT):
            ps = psum.tile([P, C], f32, tag="mm256", bufs=2)
            for f in range(NT):
                nc.tensor.matmul(
                    ps,
                    lhsT=HT[:, f, j * P : (j + 1) * P],
                    rhs=w2_s[:, f, :],
                    start=(f == 0), stop=(f == NT - 1),
                )
            outs = work.tile([P, C], f32, tag="outs")
            nc.vector.tensor_add(out=outs, in0=ps, in1=x1[:, j])
            nc.vector.tensor_add(out=outs, in0=outs, in1=b2t)
            nc.sync.dma_start(
                out=out_d[:, :].rearrange("(t p) c -> p t c", p=P)[:, j], in_=outs
            )

        for pool in (psum, work, attn, data, consts):
            pool.release()

    nc.compile()
    return nc


def _prep_inputs(x, Wq, Wk, Wv, Wp, bp, W1, b1, W2, b2, g1, be1, g2, be2):
    """Host-side preprocessing: fold LN affines into the following matmuls,
    pad per-head weights to 32-wide blocks, cast to bf16."""
    f32 = np.float32
    x = np.asarray(x, f32)
    Wqf = np.asarray(Wq, f32).reshape(C, C) * np.asarray(g1, f32)[:, None]
    Wkf = np.asarray(Wk, f32).reshape(C, C) * np.asarray(g1, f32)[:, None]
    Wvf = np.asarray(Wv, f32).reshape(C, C) * np.asarray(g1, f32)[:, None]
    bqf = np.asarray(be1, f32) @ np.asarray(Wq, f32).reshape(C, C)
    bkf = np.asarray(be1, f32) @ np.asarray(Wk, f32).reshape(C, C)
    bvf = np.asarray(be1, f32) @ np.asarray(Wv, f32).reshape(C, C)

    def pad_cols(w):
        wp = np.zeros((C, CP), f32)
        for h in range(H):
            wp[:, HP * h : HP * h + D] = w[:, D * h : D * (h + 1)]
        return wp

    def pad_vec(v):
        vp = np.zeros((CP,), f32)
        for h in range(H):
            vp[HP * h : HP * h + D] = v[D * h : D * (h + 1)]
        return vp

    wq_p = pad_cols(Wqf)
    wk_p = pad_cols(Wkf)
    wv_p = pad_cols(Wvf)
    bq_p = pad_vec(bqf)
    bk_p = pad_vec(bkf)
    bv_p = pad_vec(bvf)

    wp_p = np.zeros((CP, C), f32)
    for h in range(H):
        wp_p[HP * h : HP * h + D, :] = np.asarray(Wp, f32)[D * h : D * (h + 1), :]

    W1f = np.asarray(W1, f32) * np.asarray(g2, f32)[:, None]
    b1f = np.asarray(b1, f32) + np.asarray(be2, f32) @ np.asarray(W1, f32)

    shared = {
        "wq": wq_p.astype(_BF16), "wk": wk_p.astype(_BF16),
        "wv": wv_p.astype(_BF16), "wp": wp_p.astype(_BF16),
        "w1": W1f.astype(_BF16), "w2": np.asarray(W2, f32).astype(_BF16),
        "bq": bq_p, "bk": bk_p,
        "bprow": np.asarray(bp, f32), "b1p": b1f,
        "b2row": np.asarray(b2, f32),
    }
    assert not np.any(bv_p), "nonzero V bias not folded on-device (be1 != 0)"
    return x, shared


def kernel(**inputs) -> np.ndarray:
    from concourse import bass_utils

    x, shared = _prep_inputs(**inputs)
    if "nc" not in _cache:
        _cache["nc"] = _build_program()
    nc = _cache["nc"]

    in_maps = [dict(shared, x=np.ascontiguousarray(x[i])) for i in range(B)]
    res = bass_utils.run_bass_kernel_spmd(nc, in_maps, core_ids=list(range(B)))
    _cache["last_result"] = res
    out = np.stack([r["out"] for r in res.results], axis=0)
    return out.astype(np.float32)


# revision 13
# speedup vs baseline: 1.1593x; 1.0043x over previous
"""Trainium2 Bass kernel for one pre-LN transformer block (B=8, T=1024, C=256,
H=16 heads of size 16, FFN 256->1024->256), data-parallel over batch across 8
NeuronCores (one batch element per core).

Per-core dataflow (all matmul operands bf16, accumulation fp32):
  LN1 (straight [T,C], rstd via batched Quake-rsqrt on DVE) -> PE-transpose ->
    h1^T [C,T]
  Q^T/K^T in padded head layout [h*32+d, T] (pad rows zero, padded weights)
  V straight [T, h*32+{d,16=ones-col,zeros}] - the ones column makes the PV
    matmul also produce the softmax denominator (scores are tiny: no max pass)
  S^T[tk,tq] = k^T.T @ q^T per head via 32-row-strip matmuls (4 heads share
    the 128-row PE array), exp fused into the PSUM->SBUF copy on ScalarE
    (2 heads per activation op), causal diag blocks masked by one batched
    diagonal-strided triangular multiply per (pack, head)
  PV: out^T[d,tq] accumulated over tk tiles with 32-col-strip matmuls
  normalize via per-head Z row broadcast (stream_shuffle from PSUM) +
    reciprocal_approx_fast + multiply
  proj: x1 = x(+bp) + out^T.T @ Wp   (out^T tiles are the stationary operand)
  LN2 -> h2^T -> FFN1 (relu+bias on DVE evac) -> FFN2 -> + x1
"""

import os
import sys

for _p in ("/opt/trn_rl_repo", "/root/.axon_site/_ro/trn_rl_repo"):
    if os.path.isdir(_p) and _p not in sys.path:
        sys.path.append(_p)

import numpy as np
import ml_dtypes

# problem shapes (hardcoded per contest rules)
B, T, C, H, D, F = 8, 1024, 256, 16, 16, 1024
P = 128          # partitions
NT = T // P      # 8 T-tiles
HP = 32          # padded per-head stride (Q/K/V/out layouts)
CP = H * HP      # 512 padded channel dim
NPACK = 4        # head packs (4 heads per 128-partition tile)
NKC = C // P     # 2 k-tiles over C
EPS = 1e-5
SCALE = D ** -0.5
MAGIC = 0x5F3759DF

_BF16 = ml_dtypes.bfloat16

_cache = {}


def _build_program():
    import concourse.bass as bass
    import concourse.bacc as bacc
    import concourse.tile as tile
    import concourse.mybir as mybir

    dt = mybir.dt
    f32, bf16, i32 = dt.float32, dt.bfloat16, dt.int32
    AF = mybir.ActivationFunctionType
    ALU = mybir.AluOpType

    nc = bacc.Bacc("TRN2", target_bir_lowering=False, debug=False)

    # ---- DRAM I/O ----
    x_d = nc.dram_tensor("x", [T, C], f32, kind="ExternalInput")
    wq_d = nc.dram_tensor("wq", [C, CP], bf16, kind="ExternalInput")
    wk_d = nc.dram_tensor("wk", [C, CP], bf16, kind="ExternalInput")
    wv_d = nc.dram_tensor("wv", [C, CP], bf16, kind="ExternalInput")
    wp_d = nc.dram_tensor("wp", [CP, C], bf16, kind="ExternalInput")
    w1_d = nc.dram_tensor("w1", [C, F], bf16, kind="ExternalInput")
    w2_d = nc.dram_tensor("w2", [F, C], bf16, kind="ExternalInput")
    bq_d = nc.dram_tensor("bq", [CP], f32, kind="ExternalInput")
    bk_d = nc.dram_tensor("bk", [CP], f32, kind="ExternalInput")
    bp_d = nc.dram_tensor("bprow", [C], f32, kind="ExternalInput")
    b1_d = nc.dram_tensor("b1p", [F], f32, kind="ExternalInput")
    b2_d = nc.dram_tensor("b2row", [C], f32, kind="ExternalInput")
    out_d = nc.dram_tensor("out", [T, C], f32, kind="ExternalOutput")

    ident_np = np.eye(P, dtype=_BF16)
    # S^T diag tile mask: partition = tk local, free = tq local; keep tq >= tk
    tri_np = np.triu(np.ones((P, P), dtype=np.float32))
    tri8_np = np.tile(tri_np, (1, NT)).astype(_BF16)  # [128, 8*128]
    ident_d = nc.inline_tensor(ident_np, name="ident")
    tri8_d = nc.inline_tensor(tri8_np, name="trimask8")

    with tile.TileContext(nc) as tc:
        consts = tc.alloc_tile_pool(name="consts", bufs=1)
        data = tc.alloc_tile_pool(name="data", bufs=1)
        attn = tc.alloc_tile_pool(name="attn", bufs=1)
        work = tc.alloc_tile_pool(name="work", bufs=3)
        psum = tc.alloc_tile_pool(name="psum", bufs=1, space="PSUM")

        # ---- persistent SBUF tensors ----
        ident_s = consts.tile([P, P], bf16)
        tri8_s = consts.tile([P, NT * P], bf16)
        wq_s = consts.tile([P, NKC, CP], bf16)
        wk_s = consts.tile([P, NKC, CP], bf16)
        wv_s = consts.tile([P, NKC, CP], bf16)
        wp_s = consts.tile([P, NPACK, C], bf16)
        w1_s = consts.tile([P, NKC, F], bf16)
        w2_s = consts.tile([P, NT, C], bf16)
        bq_s = consts.tile([P, NPACK], f32)
        bk_s = consts.tile([P, NPACK], f32)
        b1_s = consts.tile([P, NT], f32)

        xs = data.tile([P, NT, C], f32)
        xbp = data.tile([P, NT, C], f32)
        h1T = data.tile([P, NKC, T], bf16)
        QT = data.tile([P, NPACK, T], bf16)
        KT = data.tile([P, NPACK, T], bf16)
        Vv = data.tile([P, NT, CP], bf16)
        OUTT = data.tile([P, NPACK, T], bf16)
        x1 = data.tile([P, NT, C], f32)
        h2T = data.tile([P, NKC, T], bf16)
        HT = data.tile([P, NT, F], bf16)

        # ---- input DMAs ----
        nc.sync.dma_start(out=ident_s, in_=ident_d[:, :])
        nc.sync.dma_start(out=tri8_s, in_=tri8_d[:, :])
        nc.sync.dma_start(out=xs, in_=x_d[:, :].rearrange("(j p) c -> p j c", p=P))
        nc.sync.dma_start(out=xbp, in_=x_d[:, :].rearrange("(j p) c -> p j c", p=P))
        # add bp (broadcast along partitions) into the residual copy
        bp_b = bass.AP(tensor=bp_d, offset=0, ap=[[0, P], [1, C]])
        bpt = consts.tile([P, C], f32)
        nc.sync.dma_start(out=bpt, in_=bp_b)
        for j in range(NT):
            nc.vector.tensor_add(out=xbp[:, j], in0=xbp[:, j], in1=bpt)
        nc.gpsimd.dma_start(out=wq_s, in_=wq_d[:, :].rearrange("(k p) c -> p k c", p=P))
        nc.gpsimd.dma_start(out=wk_s, in_=wk_d[:, :].rearrange("(k p) c -> p k c", p=P))
        nc.gpsimd.dma_start(out=wv_s, in_=wv_d[:, :].rearrange("(k p) c -> p k c", p=P))
        nc.scalar.dma_start(out=wp_s, in_=wp_d[:, :].rearrange("(k p) c -> p k c", p=P))
        nc.scalar.dma_start(out=w1_s, in_=w1_d[:, :].rearrange("(k p) c -> p k c", p=P))
        nc.scalar.dma_start(out=w2_s, in_=w2_d[:, :].rearrange("(k p) c -> p k c", p=P))
        nc.sync.dma_start(out=bq_s, in_=bq_d[:].rearrange("(m p) -> p m", p=P))
        nc.sync.dma_start(out=bk_s, in_=bk_d[:].rearrange("(m p) -> p m", p=P))
        nc.sync.dma_start(out=b1_s, in_=b1_d[:].rearrange("(m p) -> p m", p=P))
        b2t = consts.tile([P, C], f32)
        b2_b = bass.AP(tensor=b2_d, offset=0, ap=[[0, P], [1, C]])
        nc.sync.dma_start(out=b2t, in_=b2_b)

        def ln_phase(src, dst_hT, tag):
            """LayerNorm all 8 tiles of src [128, 8, 256] f32 and write the
            transposed bf16 result into dst_hT [128, 2, 1024]."""
            mvall = work.tile([P, NT, 2], f32, tag="mvall", name=f"mv_{tag}")
            for j in range(NT):
                stats = work.tile([P, 6], f32, tag="stats")
                nc.vector.bn_stats(out=stats, in_=src[:, j])
                nc.vector.bn_aggr(out=mvall[:, j], in_=stats)
            # rstd for all tiles: Quake rsqrt + 2 Newton steps (pure DVE)
            vpe = work.tile([P, NT], f32, tag="vpe", name=f"vpe_{tag}")
            nc.vector.tensor_scalar_add(out=vpe, in0=mvall[:, :, 1], scalar1=EPS)
            sh = work.tile([P, NT], i32, tag="rsq_sh")
            nc.vector.tensor_scalar(
                out=sh, in0=vpe.bitcast(i32), scalar1=1, scalar2=None,
                op0=ALU.logical_shift_right,
            )
            y0 = work.tile([P, NT], i32, tag="rsq_y0")
            nc.vector.tensor_scalar(
                out=y0, in0=sh, scalar1=-1, scalar2=MAGIC,
                op0=ALU.mult, op1=ALU.add,
            )
            y = y0.bitcast(f32)
            rsq = work.tile([P, NT], f32, tag="rsq", name=f"rsq_{tag}")
            tmp = work.tile([P, NT], f32, tag="rsq_tmp")
            for it in range(2):
                nc.vector.tensor_tensor(out=tmp, in0=y, in1=y, op=ALU.mult)
                nc.vector.tensor_tensor(out=tmp, in0=tmp, in1=vpe, op=ALU.mult)
                nc.vector.tensor_scalar(
                    out=tmp, in0=tmp, scalar1=-0.5, scalar2=1.5,
                    op0=ALU.mult, op1=ALU.add,
                )
                nc.vector.tensor_tensor(out=rsq, in0=tmp, in1=y, op=ALU.mult)
                y = rsq
            for j in range(NT):
                hs = work.tile([P, C], bf16, tag="hstraight")
                nc.vector.tensor_scalar(
                    out=hs, in0=src[:, j],
                    scalar1=mvall[:, j, 0:1], scalar2=rsq[:, j : j + 1],
                    op0=ALU.subtract, op1=ALU.mult,
                )
                tp = psum.tile([P, 2, P], bf16, tag="mm256", bufs=2)
                nc.tensor.transpose(tp[:, 0], hs[:, 0:P], ident_s)
                nc.tensor.transpose(tp[:, 1], hs[:, P : 2 * P], ident_s)
                nc.vector.tensor_copy(dst_hT[:, :, j * P : (j + 1) * P], tp)

        # ---- LN1 + h1^T ----
        ln_phase(xs, h1T, "ln1")

        # ---- Q^T / K^T (padded layout, bias folded in evac) ----
        for (name, w_s, b_s, dstT) in (("q", wq_s, bq_s, QT), ("k", wk_s, bk_s, KT)):
            for m in range(NPACK):
                for c in range(2):
                    ps = psum.tile([P, 512], f32, tag="pv", bufs=2)
                    for k in range(NKC):
                        nc.tensor.matmul(
                            ps,
                            lhsT=w_s[:, k, m * P : (m + 1) * P],
                            rhs=h1T[:, k, c * 512 : (c + 1) * 512],
                            start=(k == 0), stop=(k == NKC - 1),
                        )
                    nc.vector.tensor_scalar_add(
                        out=dstT[:, m, c * 512 : (c + 1) * 512], in0=ps,
                        scalar1=b_s[:, m : m + 1],
                    )

        # ---- V (straight, padded 32-wide blocks; col 16 of each = ones) ----
        for j in range(NT):
            ps = psum.tile([P, 512], f32, tag="pv", bufs=2)
            for k in range(NKC):
                nc.tensor.matmul(
                    ps,
                    lhsT=h1T[:, k, j * P : (j + 1) * P],
                    rhs=wv_s[:, k, :],
                    start=(k == 0), stop=(k == NKC - 1),
                )
            nc.scalar.copy(Vv[:, j, :], ps)
        ones_cols = Vv.rearrange("p j (h e) -> p j h e", e=HP)[:, :, :, 16:17]
        nc.vector.memset(ones_cols, 1.0)

        # ---- attention: unit = (pack, tq-chunk); exp buffers double-buffered
        # so PE's S^T of the next unit overlaps ACT exp / PV of this one ----
        for p in range(NPACK):
            for cj in range(2):
                expc = attn.tile([P, NPACK, NT, 512], bf16, tag="expc", bufs=2,
                                 name=f"expc{p}_{cj}")
                tiles = list(range(0, min(NT, 4 * cj + 4)))
                # S^T + exp; 2 heads share one 2-bank psum tile
                for i in tiles:
                    off = max(0, P * i - 512 * cj)  # valid start within chunk
                    n = 512 - off
                    for q in range(2):  # head pair
                        sp = psum.tile([P, 2, 512], f32, tag="sps", bufs=2,
                                       name=f"sp{p}_{cj}_{i}_{q}")
                        for e in range(2):
                            hh = 2 * q + e
                            nc.tensor.matmul(
                                sp[:, e, 0:n],
                                lhsT=KT[HP * hh : HP * (hh + 1), p,
                                        i * P : (i + 1) * P],
                                rhs=QT[HP * hh : HP * (hh + 1), p,
                                       512 * cj + off : 512 * cj + off + n],
                                start=True, stop=True,
                                tile_position=(HP * hh, 0),
                            )
                        nc.scalar.activation(
                            out=expc[:, 2 * q : 2 * q + 2, i, off : off + n],
                            in_=sp[:, :, 0:n],
                            func=AF.Exp, scale=SCALE,
                        )
                # causal mask: the 4 diagonal blocks of this chunk per head
                for hh in range(NPACK):
                    base = expc[:, hh]
                    dview = bass.AP(
                        tensor=base.tensor,
                        offset=base.offset + 2048 * cj,
                        ap=[list(base.ap[0]), [512 + P, 4], [1, P]],
                    )
                    nc.vector.tensor_tensor(
                        out=dview, in0=dview,
                        in1=tri8_s[:, 0 : 4 * P].rearrange(
                            "p (j q) -> p j q", q=P
                        ),
                        op=ALU.mult,
                    )
                # PV accumulation over valid tk tiles
                pv = psum.tile([P, 512], f32, tag="pv", bufs=2,
                               name=f"pv{p}_{cj}")
                last = max(tiles)
                for i in tiles:
                    off = max(0, P * i - 512 * cj)
                    n = 512 - off
                    for hh in range(NPACK):
                        h = 4 * p + hh
                        nc.tensor.matmul(
                            pv[HP * hh : HP * (hh + 1), off : off + n],
                            lhsT=Vv[:, i, HP * h : HP * (h + 1)],
                            rhs=expc[:, hh, i, off : off + n],
                            start=(i == 0), stop=(i == last),
                            tile_position=(0, HP * hh),
                            skip_group_check=True,
                        )
                # normalize: out^T = pv / Z  (Z in partition 16 of each 32-block)
                zbc = work.tile([P, 512], f32, tag="zbc")
                rz = work.tile([P, 512], f32, tag="rz")
                nc.vector.stream_shuffle(zbc, pv, mask=[16] * 32)
                nc.vector.reciprocal_approx_fast(out=rz, in_=zbc)
                nc.vector.tensor_tensor(
                    out=OUTT[:, p, 512 * cj : 512 * (cj + 1)], in0=pv, in1=rz,
                    op=ALU.mult,
                )

        # ---- attention out-projection + residual ----
        for j in range(NT):
            ps = psum.tile([P, C], f32, tag="mm256", bufs=2)
            for k in range(NPACK):
                nc.tensor.matmul(
                    ps,
                    lhsT=OUTT[:, k, j * P : (j + 1) * P],
                    rhs=wp_s[:, k, :],
                    start=(k == 0), stop=(k == NPACK - 1),
                )
            nc.vector.tensor_add(out=x1[:, j], in0=ps, in1=xbp[:, j])

        # ---- LN2 + h2^T ----
        ln_phase(x1, h2T, "ln2")

        # ---- FFN1: H^T = relu(W1^T h2^T + b1) on DVE evac ----
        for f in range(NT):
            for c in range(2):
                ps = psum.tile([P, 512], f32, tag="pv", bufs=2)
                for k in range(NKC):
                    nc.tensor.matmul(
                        ps,
                        lhsT=w1_s[:, k, f * P : (f + 1) * P],
                        rhs=h2T[:, k, c * 512 : (c + 1) * 512],
                        start=(k == 0), stop=(k == NKC - 1),
                    )
                nc.vector.tensor_scalar(
                    out=HT[:, f, c * 512 : (c + 1) * 512], in0=ps,
                    scalar1=b1_s[:, f : f + 1], scalar2=0.0,
                    op0=ALU.add, op1=ALU.max,
                )

        # ---- FFN2 + residual + b2, write out ----
        for j in range(NT):
            ps = psum.tile([P, C], f32, tag="mm256", bufs=2)
            for f in range(NT):
                nc.tensor.matmul(
                    ps,
                    lhsT=HT[:, f, j * P : (j + 1) * P],
                    rhs=w2_s[:, f, :],
                    start=(f == 0), stop=(f == NT - 1),
                )
            outs = work.tile([P, C], f32, tag="outs")
            nc.vector.tensor_add(out=outs, in0=ps, in1=x1[:, j])
            nc.vector.tensor_add(out=outs, in0=outs, in1=b2t)
            nc.sync.dma_start(
                out=out_d[:, :].rearrange("(t p) c -> p t c", p=P)[:, j], in_=outs
            )

        for pool in (psum, work, attn, data, consts):
            pool.release()

    nc.compile()
    return nc


def _prep_inputs(x, Wq, Wk, Wv, Wp, bp, W1, b1, W2, b2, g1, be1, g2, be2):
    """Host-side preprocessing: fold LN affines into the following matmuls,
    pad per-head weights to 32-wide blocks, cast to bf16."""
    f32 = np.float32
    x = np.asarray(x, f32)
    Wqf = np.asarray(Wq, f32).reshape(C, C) * np.asarray(g1, f32)[:, None]
    Wkf = np.asarray(Wk, f32).reshape(C, C) * np.asarray(g1, f32)[:, None]
    Wvf = np.asarray(Wv, f32).reshape(C, C) * np.asarray(g1, f32)[:, None]
    bqf = np.asarray(be1, f32) @ np.asarray(Wq, f32).reshape(C, C)
    bkf = np.asarray(be1, f32) @ np.asarray(Wk, f32).reshape(C, C)
    bvf = np.asarray(be1, f32) @ np.asarray(Wv, f32).reshape(C, C)

    def pad_cols(w):
        wp = np.zeros((C, CP), f32)
        for h in range(H):
            wp[:, HP * h : HP * h + D] = w[:, D * h : D * (h + 1)]
        return wp

    def pad_vec(v):
        vp = np.zeros((CP,), f32)
        for h in range(H):
            vp[HP * h : HP * h + D] = v[D * h : D * (h + 1)]
        return vp

    wq_p = pad_cols(Wqf)
    wk_p = pad_cols(Wkf)
    wv_p = pad_cols(Wvf)
    bq_p = pad_vec(bqf)
    bk_p = pad_vec(bkf)
    bv_p = pad_vec(bvf)

    wp_p = np.zeros((CP, C), f32)
    for h in range(H):
        wp_p[HP * h : HP * h + D, :] = np.asarray(Wp, f32)[D * h : D * (h + 1), :]

    W1f = np.asarray(W1, f32) * np.asarray(g2, f32)[:, None]
    b1f = np.asarray(b1, f32) + np.asarray(be2, f32) @ np.asarray(W1, f32)

    shared = {
        "wq": wq_p.astype(_BF16), "wk": wk_p.astype(_BF16),
        "wv": wv_p.astype(_BF16), "wp": wp_p.astype(_BF16),
        "w1": W1f.astype(_BF16), "w2": np.asarray(W2, f32).astype(_BF16),
        "bq": bq_p, "bk": bk_p,
        "bprow": np.asarray(bp, f32), "b1p": b1f,
        "b2row": np.asarray(b2, f32),
    }
    assert not np.any(bv_p), "nonzero V bias not folded on-device (be1 != 0)"
    return x, shared


def kernel(**inputs) -> np.ndarray:
    from concourse import bass_utils

    x, shared = _prep_inputs(**inputs)
    if "nc" not in _cache:
        _cache["nc"] = _build_program()
    nc = _cache["nc"]

    in_maps = [dict(shared, x=np.ascontiguousarray(x[i])) for i in range(B)]
    res = bass_utils.run_bass_kernel_spmd(nc, in_maps, core_ids=list(range(B)))
    _cache["last_result"] = res
    out = np.stack([r["out"] for r in res.results], axis=0)
    return out.astype(np.float32)
